# revision 16
# baseline (speedup 1.0000x reference)
"""GIN (3-layer) message-passing kernel for Trainium2, 8 NeuronCores.

Strategy (graph-partition data parallel):
  - Graphs (variable 49-51 nodes, from the `batch` input) are padded to
    GP=52 columns each.  Core c owns graphs [750c, 750(c+1)), i.e. 39000
    padded columns (SHP=39040 with block padding); every graph lives on
    exactly one core so segment-max pooling is core-local.
  - Edges sharded by destination core.  Host sorts each core's edges by
    (512-column destination group, padded local dst) and pads each GROUP's
    edge list to a multiple of 128 ("k-tiles").  The k-tile structure is
    shared across cores (max over cores per group) so the SPMD program is
    identical on all cores.  Self-loops are NOT gathered: the GIN "+ h_i"
    term is added as a (pre-masked) column slab of the previous layer's z.
  - Aggregation: per-k-tile indirect-DMA gathers fetch h[src] rows (256B
    fp16) from a replicated DRAM table; a per-k-tile one-hot matrix S
    [128 x 512] (fp16, generated on the vector engine with iota/is_equal
    against rel = dst-in-group) right-multiplies the gathered tile on the
    tensor engine, accumulating aggT[feat, col] in a full-bank PSUM tile
    per group.
  - MLP runs in transposed space (feat on partitions).  BatchNorm of the
    previous layer is folded into the next layer's first matmul (w1
    row-scaled by s, plus a rank-1 (w1^T t) x (deg+1) correction), so h
    tables stay un-normalized.
  - Pad columns see zero aggregation and zero deg, so their z2 value is the
    input-independent constant v_l = relu(w2^T relu(b1) + b2).  BN stats
    (from activation accum_out, taken before masking) are corrected by
    npad*[v, v^2] before the 1KB AllReduce.  z2 is then multiplied by the
    0/1 column mask, so pad columns are exactly 0 in zT / z_rm / h_tbl --
    making the pooling max (z2 >= 0) and the next layer's self-term
    correct without further masking.
  - AllGather (fp16, 10MB -> 80MB) rebuilds the replicated h table between
    layers.
  - Pooling: per-layer local segment-max (fixed 52-wide windows) over the
    fp32 zT table, then the (monotone, gamma>0) BN affine, transpose,
    concat and per-core output.
Host assembles the 8 per-core [750, 384] outputs into the full [6000, 384].
"""

import sys

sys.path.insert(0, "/opt/trn_rl_repo")

from dataclasses import dataclass

import numpy as np


@dataclass(frozen=True)
class Cfg:
    n_nodes: int = 300000
    n_graphs: int = 6000
    n_cores: int = 8
    in_dim: int = 77
    dim: int = 128
    gp: int = 52  # padded columns per graph
    grp_blks: int = 4  # 128-col blocks per MLP/agg group (=512 cols)
    eps: float = 1e-5

    @property
    def gpc(self):  # graphs per core
        return self.n_graphs // self.n_cores

    @property
    def shc(self):  # padded columns per core (graph padding only)
        return self.gpc * self.gp

    @property
    def nb(self):  # 128-col blocks per core
        return (self.shc + 127) // 128

    @property
    def shp(self):  # block-padded columns per core
        return self.nb * 128

    @property
    def tbl(self):  # replicated table rows
        return self.n_cores * self.shp

    @property
    def ng(self):  # groups per core
        return (self.nb + self.grp_blks - 1) // self.grp_blks


@dataclass
class HostData:
    kt_total: int
    grp_kt0: np.ndarray  # [ng] int, first k-tile of each group
    grp_nk: np.ndarray  # [ng] int, k-tiles per group
    idx_sb: list  # per core [128, KT] int32 gather row ids
    rel_sb: list  # per core [128, KT] f32 dst-in-group (or -1 pad)
    deg2: list  # per core [128, ncolg*GW] f32 (packed rows 0/64)
    xT: list  # per core [128, SHP] f32 transposed padded x
    x_tbl: np.ndarray  # [tbl, 128] f16


def prep_host(cfg: Cfg, x: np.ndarray, edge_index: np.ndarray,
              batch: np.ndarray, inputs: dict) -> HostData:
    C, NG, SHP = cfg.n_cores, cfg.ng, cfg.shp
    N, GP, GPC = cfg.n_nodes, cfg.gp, cfg.gpc
    GW = cfg.grp_blks * 128
    batch = batch.astype(np.int64)
    sizes = np.bincount(batch, minlength=cfg.n_graphs)
    assert sizes.max() <= GP, f"graph size {sizes.max()} > pad {GP}"
    gstart = np.zeros(cfg.n_graphs + 1, dtype=np.int64)
    np.cumsum(sizes, out=gstart[1:])
    rank = np.arange(N, dtype=np.int64) - gstart[batch]
    padpos = (batch - (batch // GPC) * GPC) * GP + rank  # within-core col
    ncore = batch // GPC  # owning core of each node
    row = ncore * SHP + padpos  # replicated-table row

    src = edge_index[0].astype(np.int64)
    dst = edge_index[1].astype(np.int64)
    dcore = ncore[dst]
    dloc = padpos[dst]

    per_core = []
    cnts = np.zeros((C, NG), dtype=np.int64)
    for c in range(C):
        m = dcore == c
        s_c, dl_c = src[m], dloc[m]
        order = np.argsort(dl_c, kind="stable")
        s_c, dl_c = s_c[order], dl_c[order]
        grp = dl_c // GW
        cnts[c] = np.bincount(grp, minlength=NG)
        per_core.append((s_c, dl_c, grp))

    grp_nk = (cnts.max(axis=0) + 127) // 128  # shared k-tile structure
    grp_nk = np.maximum(grp_nk, 1)
    grp_kt0 = np.concatenate([[0], np.cumsum(grp_nk)[:-1]])
    KT = int(grp_nk.sum())
    k_pad = KT * 128

    idx_sb, rel_sb, deg2, xT = [], [], [], []
    for c in range(C):
        s_c, dl_c, grp = per_core[c]
        gs = np.concatenate([[0], np.cumsum(cnts[c])[:-1]])
        pos = np.arange(len(s_c)) - gs[grp]
        slot = grp_kt0[grp] * 128 + pos
        idx_arr = np.zeros(k_pad, dtype=np.int32)
        rel_arr = np.full(k_pad, -1.0, dtype=np.float32)
        idx_arr[slot] = row[s_c].astype(np.int32)
        rel_arr[slot] = (dl_c - grp * GW).astype(np.float32)
        idx_sb.append(np.ascontiguousarray(idx_arr.reshape(KT, 128).T))
        rel_sb.append(np.ascontiguousarray(rel_arr.reshape(KT, 128).T))

        own = ncore == c
        # in-degree + 1 (self) at padded cols; row g of deg2 = group g
        indeg = np.zeros(SHP, dtype=np.float32)
        np.add.at(indeg, padpos[dst[dcore == c]], 1.0)
        realm = np.zeros(SHP, dtype=np.float32)
        realm[padpos[own]] = 1.0
        deg_p = (indeg + 1.0) * realm
        ncolg = (NG + 2) // 3
        d2 = np.zeros((128, ncolg * GW), dtype=np.float32)
        for g in range(NG):
            sl = slice(g * GW, min((g + 1) * GW, SHP))
            n = sl.stop - sl.start
            d2[(g % 3) * 32, (g // 3) * GW: (g // 3) * GW + n] = deg_p[sl]
        deg2.append(d2)
        xt = np.zeros((128, SHP), dtype=np.float32)
        xt[: cfg.in_dim, padpos[own]] = x[own].T
        xT.append(np.ascontiguousarray(xt))

    x_tbl = np.zeros((cfg.tbl, 128), dtype=np.float16)
    x_tbl[row, : cfg.in_dim] = x.astype(np.float16)
    return HostData(KT, grp_kt0, grp_nk, idx_sb, rel_sb, deg2, xT, x_tbl)


def build_program(cfg: Cfg, hd: HostData, weights_np: dict):
    """Returns (nc, input_names).  weights_np supplies shapes only."""
    import concourse.bass as bass
    import concourse.mybir as mybir
    import concourse.tile as tile
    from concourse import bacc
    from concourse.masks import make_identity

    dt = mybir.dt
    Alu = mybir.AluOpType
    Act = mybir.ActivationFunctionType

    C, D, NB, SHP, TBL, NG, KT = (
        cfg.n_cores, cfg.dim, cfg.nb, cfg.shp, cfg.tbl, cfg.ng, hd.kt_total,
    )
    GW = cfg.grp_blks * 128  # group width (cols)
    GS, GPC = cfg.gp, cfg.gpc
    inv_n = 1.0 / cfg.n_nodes

    nc = bacc.Bacc(
        "TRN2", target_bir_lowering=False, debug=False, num_devices=C
    )

    def din(name, shape, dtp=dt.float32):
        return nc.dram_tensor(name, list(shape), dtp, kind="ExternalInput").ap()

    x_tbl_d = din("x_tbl", (TBL, D), dt.float16)
    idx_d = din("idx", (128, KT), dt.int32)
    rel_d = din("rel", (128, KT))
    ncolg = (NG + 2) // 3
    deg2_d = din("deg2", (128, ncolg * GW))
    xT_d = din("xT", (128, SHP))
    iota_d = din("iota", (128, GW), dt.float16)
    w1_d = [din(f"w1_{l}", (D, D)) for l in range(3)]
    w2_d = [din(f"w2_{l}", (D, D)) for l in range(3)]
    b1_d = [din(f"b1_{l}", (D, 1)) for l in range(3)]
    b2_d = [din(f"b2_{l}", (D, 1)) for l in range(3)]
    gb_d = din("gb", (D, 6))  # cols: g0 b0 g1 b1 g2 b2
    out_d = nc.dram_tensor(
        "pooled", [GPC, 3 * D], dt.float32, kind="ExternalOutput"
    ).ap()

    input_names = (
        ["x_tbl", "idx", "rel", "deg2", "xT", "iota"]
        + [f"w1_{l}" for l in range(3)]
        + [f"w2_{l}" for l in range(3)]
        + [f"b1_{l}" for l in range(3)]
        + [f"b2_{l}" for l in range(3)]
        + ["gb"]
    )

    n_pool_chunks = (GPC + 127) // 128
    last_chunk_rows = GPC - (n_pool_chunks - 1) * 128
    # pooling slab: whole graphs per slab, tile cols = graphs*GS
    slab_graphs = max(1, 2496 // GS)
    n_slabs = (GPC + slab_graphs - 1) // slab_graphs

    with tile.TileContext(nc) as tc:
        with (
            tc.tile_pool(name="const", bufs=1) as cpool,
            tc.tile_pool(name="ebuf", bufs=8) as epool,
            tc.tile_pool(name="spool", bufs=4) as spool,
            tc.tile_pool(name="zin", bufs=2) as zinpool,
            tc.tile_pool(name="zmid", bufs=2) as zmidpool,
            tc.tile_pool(name="slfp", bufs=2) as slfpool,
            tc.tile_pool(name="rm", bufs=3) as rmpool,
            tc.tile_pool(name="stat", bufs=1) as statpool,
            tc.tile_pool(name="slab", bufs=2) as slabpool,
            tc.tile_pool(name="poolt", bufs=2) as ptpool,
            tc.tile_pool(name="agg_ps", bufs=2, space="PSUM") as aggpool,
            tc.tile_pool(name="m1_ps", bufs=2, space="PSUM") as m1pool,
            tc.tile_pool(name="m2_ps", bufs=2, space="PSUM") as m2pool,
            tc.tile_pool(name="tr_ps", bufs=2, space="PSUM") as trpool,
            tc.tile_pool(name="dram", bufs=1, space="DRAM") as dpool,
        ):
            # ---- DRAM intermediates ----
            h_tbl = [
                dpool.tile(
                    [TBL, D], dt.float16, name=f"h_tbl{l}", addr_space="Shared"
                )
                for l in range(2)
            ]
            z_rm = dpool.tile([SHP, D], dt.float16, name="z_rm")
            zT = [
                dpool.tile([D, SHP], dt.float32, name=f"zT{l}") for l in range(3)
            ]
            st_in = [
                dpool.tile([D, 2], dt.float32, name=f"st_in{l}") for l in range(3)
            ]
            st_out = [
                dpool.tile([D, 2], dt.float32, name=f"st_out{l}")
                for l in range(3)
            ]

            # ---- constants to SBUF ----
            def load(shape, src_ap, dtp=dt.float32, name=None):
                t = cpool.tile(list(shape), dtp, name=name)
                nc.sync.dma_start(out=t[:], in_=src_ap)
                return t

            idx_sb = load((128, KT), idx_d[:], dt.int32, name="idx_sb")
            rel_sb = load((128, KT), rel_d[:], name="rel_sb")
            deg2_sb = load((128, ncolg * GW), deg2_d[:], name="deg2_sb")
            # pad-col indicator: deg2 is deg+1>=1 at real cols, 0 at pads
            padm2_sb = cpool.tile([128, ncolg * GW], dt.float32, name="padm2_sb")
            nc.vector.tensor_scalar(
                out=padm2_sb[:], in0=deg2_sb[:], scalar1=0.0, scalar2=None,
                op0=Alu.is_equal,
            )
            iota_sb = load((128, GW), iota_d[:], dt.float16, name="iota_sb")
            w1_sb = [load((D, D), w1_d[l][:], name=f"w1sb{l}") for l in range(3)]
            w2_sb = [load((D, D), w2_d[l][:], name=f"w2sb{l}") for l in range(3)]
            b1_sb = [load((D, 1), b1_d[l][:], name=f"b1sb{l}") for l in range(3)]
            b2_sb = [load((D, 1), b2_d[l][:], name=f"b2sb{l}") for l in range(3)]
            gb_sb = load((D, 6), gb_d[:], name="gb_sb")
            ident = cpool.tile([128, 128], dt.float32, name="ident")
            make_identity(nc, ident[:])

            # persistent small tiles
            s_all = cpool.tile([D, 3], dt.float32, name="s_all")
            t_all = cpool.tile([D, 3], dt.float32, name="t_all")
            w1s_sb = [
                cpool.tile([D, D], dt.float32, name=f"w1s{l}") for l in (1, 2)
            ]
            u_sb = [cpool.tile([1, D], dt.float32, name=f"u{l}") for l in (1, 2)]
            ub_sb = [
                cpool.tile([D, D], dt.float32, name=f"ub{l}") for l in (1, 2)
            ]
            ones_row = cpool.tile([1, D], dt.float32, name="ones_row")
            nc.gpsimd.memset(ones_row[:], 1.0)
            negbig = cpool.tile([128, D], dt.float32, name="negbig")
            nc.gpsimd.memset(negbig[:], -1e30)
            ssum = cpool.tile([128, NG], dt.float32, name="ssum")
            ssq = cpool.tile([128, NG], dt.float32, name="ssq")
            sq_scr = cpool.tile([128, GW], dt.float32, name="sq_scr")
            stat_scr = cpool.tile([128, 8], dt.float32, name="stat_scr")

            def compute_fold(l):
                """Load layer-l AR'd stats; fill s_all/t_all col l and (for
                l<2) w1s_sb/u_sb of layer l+1."""
                st = statpool.tile([D, 2], dt.float32, name="st_ld")
                nc.sync.dma_start(out=st[:], in_=st_out[l][:])
                mu = stat_scr[:, 0:1]
                msq = stat_scr[:, 1:2]
                var = stat_scr[:, 2:3]
                rstd = stat_scr[:, 3:4]
                smu = stat_scr[:, 4:5]
                nc.vector.tensor_scalar_mul(mu, st[:, 0:1], inv_n)
                nc.vector.tensor_scalar_mul(msq, st[:, 1:2], inv_n)
                nc.vector.tensor_tensor(
                    out=var, in0=mu, in1=mu, op=Alu.mult
                )
                nc.vector.tensor_tensor(
                    out=var, in0=msq, in1=var, op=Alu.subtract
                )
                veps = stat_scr[:, 6:7]
                nc.vector.tensor_scalar_add(veps, var, cfg.eps)
                std = stat_scr[:, 5:6]
                nc.scalar.activation(std, veps, Act.Sqrt)
                nc.vector.reciprocal(rstd, std)
                scol = s_all[:, l : l + 1]
                tcol = t_all[:, l : l + 1]
                nc.vector.tensor_tensor(
                    out=scol, in0=gb_sb[:, 2 * l : 2 * l + 1], in1=rstd,
                    op=Alu.mult,
                )
                nc.vector.tensor_tensor(out=smu, in0=scol, in1=mu, op=Alu.mult)
                nc.vector.tensor_tensor(
                    out=tcol, in0=gb_sb[:, 2 * l + 1 : 2 * l + 2], in1=smu,
                    op=Alu.subtract,
                )
                if l < 2:
                    ln = l + 1
                    nc.vector.tensor_scalar(
                        out=w1s_sb[ln - 1][:], in0=w1_sb[ln][:], scalar1=scol,
                        scalar2=None, op0=Alu.mult,
                    )
                    ups = trpool.tile([1, D], dt.float32, name="ups", tag="tr")
                    nc.tensor.matmul(
                        ups[:], lhsT=tcol, rhs=w1_sb[ln][:], start=True,
                        stop=True,
                    )
                    nc.any.tensor_copy(out=u_sb[ln - 1][:], in_=ups[:])
                    ubp = trpool.tile([D, D], dt.float32, name="ubp", tag="tr")
                    nc.tensor.matmul(
                        ubp[:], lhsT=ones_row[:], rhs=u_sb[ln - 1][:],
                        start=True, stop=True,
                    )
                    nc.any.tensor_copy(out=ub_sb[ln - 1][:], in_=ubp[:])

            out_big = cpool.tile(
                [128, n_pool_chunks * 3 * D], dt.float32, name="out_big"
            )

            def emit_pool(l):
                """Segment-max pool layer l's zT, BN-affine, transpose into
                out_big.  Requires compute_fold(l) done (s_all/t_all col l)."""
                pt = ptpool.tile([128, GPC], dt.float32, name="pt")
                for si in range(n_slabs):
                    g0 = si * slab_graphs
                    g1 = min(g0 + slab_graphs, GPC)
                    ncols = (g1 - g0) * GS
                    sl = slabpool.tile(
                        [128, slab_graphs * GS], dt.float32, name="sl"
                    )
                    nc.sync.dma_start(
                        out=sl[:, :ncols],
                        in_=zT[l][:, g0 * GS : g0 * GS + ncols],
                    )
                    nc.vector.tensor_reduce(
                        out=pt[:, g0:g1],
                        in_=sl[:, :ncols].rearrange("p (g s) -> p g s", s=GS),
                        axis=mybir.AxisListType.X, op=Alu.max,
                    )
                pta = ptpool.tile([128, GPC], dt.float32, name="pta")
                nc.vector.tensor_scalar(
                    out=pta[:], in0=pt[:], scalar1=s_all[:, l : l + 1],
                    scalar2=t_all[:, l : l + 1], op0=Alu.mult, op1=Alu.add,
                )
                for ch in range(n_pool_chunks):
                    rows = 128 if ch < n_pool_chunks - 1 else last_chunk_rows
                    trp = trpool.tile(
                        [128, 128], dt.float32, name="trpo", tag="tr"
                    )
                    nc.tensor.transpose(
                        trp[:rows, :], pta[:, ch * 128 : ch * 128 + rows],
                        ident[:],
                    )
                    nc.any.tensor_copy(
                        out=out_big[
                            :rows,
                            ch * 3 * D + l * D : ch * 3 * D + (l + 1) * D,
                        ],
                        in_=trp[:rows, :],
                    )

            for layer in range(3):
                tbl_ap = x_tbl_d if layer == 0 else h_tbl[layer - 1][:]
                if layer > 0:
                    compute_fold(layer - 1)
                    emit_pool(layer - 1)
                lhs1 = w1_sb[0] if layer == 0 else w1s_sb[layer - 1]

                for g in range(NG):
                    W = min(GW, SHP - g * GW)
                    nk = int(hd.grp_nk[g])
                    t0 = int(hd.grp_kt0[g])
                    agg = aggpool.tile([128, GW], dt.float32, name="agg")
                    for j in range(nk):
                        t = t0 + j
                        esl = epool.tile([128, 128], dt.float16, name="ebuf")
                        nc.gpsimd.indirect_dma_start(
                            out=esl[:],
                            out_offset=None,
                            in_=tbl_ap,
                            in_offset=bass.IndirectOffsetOnAxis(
                                ap=idx_sb[:, t : t + 1], axis=0,
                            ),
                        )
                        s_t = spool.tile([128, GW], dt.float16, name="s_t")
                        nc.vector.tensor_scalar(
                            out=s_t[:, :W], in0=iota_sb[:, :W],
                            scalar1=rel_sb[:, t : t + 1],
                            scalar2=None, op0=Alu.is_equal,
                        )
                        nc.tensor.matmul(
                            agg[:, :W], lhsT=esl[:], rhs=s_t[:, :W],
                            start=(j == 0), stop=(j == nk - 1),
                        )
                    # ---- self term: zin = agg + h_prev (pre-masked) ----
                    zin = zinpool.tile([128, GW], dt.float32, name="zin")
                    slf = slfpool.tile([128, GW], dt.float32, name="slf")
                    if layer == 0:
                        nc.sync.dma_start(
                            out=slf[:, :W], in_=xT_d[:, g * GW : g * GW + W]
                        )
                    else:
                        nc.sync.dma_start(
                            out=slf[:, :W],
                            in_=zT[layer - 1][:, g * GW : g * GW + W],
                        )
                    nc.vector.tensor_tensor(
                        out=zin[:, :W], in0=agg[:, :W], in1=slf[:, :W],
                        op=Alu.add,
                    )
                    # ---- MLP on the group (transposed space) ----
                    m1 = m1pool.tile([128, GW], dt.float32, name="m1")
                    nc.tensor.matmul(
                        m1[:, :W], lhsT=lhs1[:], rhs=zin[:, :W],
                        start=True, stop=(layer == 0),
                    )
                    if layer > 0:
                        dp1 = (g % 3) * 32
                        dc1 = (g // 3) * GW
                        nc.tensor.matmul(
                            m1[:, :W], lhsT=ub_sb[layer - 1][dp1 : dp1 + 1, :],
                            rhs=deg2_sb[dp1 : dp1 + 1, dc1 : dc1 + W],
                            start=False, stop=True,
                        )
                    z1 = zmidpool.tile([128, GW], dt.float32, name="z1")
                    nc.scalar.activation(
                        z1[:, :W], m1[:, :W], Act.Relu, bias=b1_sb[layer][:]
                    )
                    m2 = m2pool.tile([128, GW], dt.float32, name="m2")
                    nc.tensor.matmul(
                        m2[:, :W], lhsT=w2_sb[layer][:], rhs=z1[:, :W],
                        start=True, stop=False,
                    )
                    dp = (g % 3) * 32
                    dc = (g // 3) * GW
                    nc.tensor.matmul(
                        m2[:, :W], lhsT=negbig[dp : dp + 1, :],
                        rhs=padm2_sb[dp : dp + 1, dc : dc + W],
                        start=False, stop=True,
                    )
                    z2 = zmidpool.tile([128, GW], dt.float32, name="z2")
                    nc.scalar.activation(
                        z2[:, :W], m2[:, :W], Act.Relu,
                        bias=b2_sb[layer][:], accum_out=ssum[:, g : g + 1],
                    )
                    nc.scalar.activation(
                        sq_scr[:, :W], z2[:, :W], Act.Square,
                        accum_out=ssq[:, g : g + 1],
                    )
                    nc.sync.dma_start(
                        out=zT[layer][:, g * GW : g * GW + W], in_=z2[:, :W]
                    )
                    if layer < 2:
                        rm = rmpool.tile([128, GW], dt.float16, name="rm")
                        for i in range(W // 128):
                            trp = trpool.tile(
                                [128, 128], dt.float32, name="trp", tag="tr"
                            )
                            nc.tensor.transpose(
                                trp[:], z2[:, i * 128 : (i + 1) * 128],
                                ident[:],
                            )
                            nc.any.tensor_copy(
                                out=rm[:, i * 128 : (i + 1) * 128], in_=trp[:]
                            )
                        nc.sync.dma_start(
                            out=z_rm[g * GW : g * GW + W, :].rearrange(
                                "(k p) d -> p k d", p=128
                            ),
                            in_=rm[:, :W].rearrange("p (k d) -> p k d", d=D),
                        )

                # ---- stats reduce (pad-corrected) + AllReduce ----
                sp = statpool.tile([D, 2], dt.float32, name="sp")
                nc.vector.tensor_reduce(
                    out=sp[:, 0:1], in_=ssum[:, :NG],
                    axis=mybir.AxisListType.X, op=Alu.add,
                )
                nc.vector.tensor_reduce(
                    out=sp[:, 1:2], in_=ssq[:, :NG],
                    axis=mybir.AxisListType.X, op=Alu.add,
                )
                nc.sync.dma_start(out=st_in[layer][:], in_=sp[:])
                nc.gpsimd.collective_compute(
                    "AllReduce", Alu.add,
                    replica_groups=[list(range(C))],
                    ins=[st_in[layer].opt()], outs=[st_out[layer].opt()],
                )
                if layer < 2:
                    nc.gpsimd.collective_compute(
                        "AllGather", Alu.bypass,
                        replica_groups=[list(range(C))],
                        ins=[z_rm.opt()], outs=[h_tbl[layer].opt()],
                    )

            # ---- final fold + pool of layer 2, then output ----
            compute_fold(2)
            emit_pool(2)
            for ch in range(n_pool_chunks):
                rows = 128 if ch < n_pool_chunks - 1 else last_chunk_rows
                nc.sync.dma_start(
                    out=out_d[ch * 128 : ch * 128 + rows, :],
                    in_=out_big[:rows, ch * 3 * D : (ch + 1) * 3 * D],
                )

    nc.compile()
    return nc, input_names


def make_in_maps(cfg: Cfg, hd: HostData, inputs: dict, input_names):
    GW = cfg.grp_blks * 128
    # column index 0..GW-1 (dst-in-group), same on every partition
    iota = np.tile(np.arange(GW, dtype=np.float16), (128, 1))
    gb = np.zeros((cfg.dim, 6), dtype=np.float32)
    for l in range(3):
        gb[:, 2 * l] = inputs["gamma"][l]
        gb[:, 2 * l + 1] = inputs["beta"][l]
    w1p = []
    for l in range(3):
        w = np.zeros((cfg.dim, cfg.dim), dtype=np.float32)
        wl = inputs[f"w1_{l}"]
        w[: wl.shape[0], :] = wl
        w1p.append(w)
    shared = {
        "x_tbl": hd.x_tbl,
        "iota": np.ascontiguousarray(iota),
        "gb": gb,
    }
    for l in range(3):
        shared[f"w1_{l}"] = w1p[l]
        shared[f"w2_{l}"] = np.ascontiguousarray(inputs[f"w2_{l}"].astype(np.float32))
        shared[f"b1_{l}"] = inputs[f"b1_{l}"].astype(np.float32).reshape(-1, 1)
        shared[f"b2_{l}"] = inputs[f"b2_{l}"].astype(np.float32).reshape(-1, 1)
    in_maps = []
    for c in range(cfg.n_cores):
        m = dict(shared)
        m["idx"] = hd.idx_sb[c]
        m["rel"] = hd.rel_sb[c]
        m["deg2"] = hd.deg2[c]
        m["xT"] = hd.xT[c]
        assert set(m.keys()) == set(input_names)
        in_maps.append(m)
    return in_maps


def _run_sharded_timed(nc, in_maps, n_cores, iters=10, warmup=2):
    """Execute the compiled Bass module via PJRT with device-resident inputs,
    timing `iters` back-to-back dispatches (excludes input upload/compile)."""
    import time

    import jax
    from jax.sharding import Mesh, NamedSharding, PartitionSpec
    from jax.experimental.shard_map import shard_map

    import concourse.mybir as mybir
    from concourse import bass2jax

    bass2jax.install_neuronx_cc_hook()
    partition_name = (
        nc.partition_id_tensor.name if nc.partition_id_tensor else None
    )
    in_names, out_names, out_avals, zero_outs = [], [], [], []
    for alloc in nc.m.functions[0].allocations:
        if not isinstance(alloc, mybir.MemoryLocationSet):
            continue
        name = alloc.memorylocations[0].name
        if alloc.kind == "ExternalInput":
            if name != partition_name:
                in_names.append(name)
        elif alloc.kind == "ExternalOutput":
            out_names.append(name)
            shape = tuple(alloc.tensor_shape)
            dtp = mybir.dt.np(alloc.dtype)
            out_avals.append(jax.core.ShapedArray(shape, dtp))
            zero_outs.append(np.zeros(shape, dtp))
    n_params, n_outs = len(in_names), len(out_avals)
    in_names.extend(out_names)
    if partition_name is not None:
        in_names.append(partition_name)
    donate = tuple(range(n_params, n_params + n_outs))

    def _body(*args):
        operands = list(args)
        if partition_name is not None:
            operands.append(bass2jax.partition_id_tensor())
        outs = bass2jax._bass_exec_p.bind(
            *operands,
            out_avals=tuple(out_avals),
            in_names=tuple(in_names),
            out_names=tuple(out_names),
            lowering_input_output_aliases=(),
            sim_require_finite=True,
            sim_require_nnan=True,
            nc=nc,
        )
        return tuple(outs)

    devices = jax.devices()[:n_cores]
    mesh = Mesh(np.asarray(devices), ("core",))
    pspec = PartitionSpec("core")
    in_specs = (pspec,) * (n_params + n_outs)
    sharded = jax.jit(
        shard_map(
            _body, mesh=mesh, in_specs=in_specs,
            out_specs=(pspec,) * len(out_names), check_rep=False,
        ),
        donate_argnums=donate, keep_unused=True,
    )
    shd = NamedSharding(mesh, pspec)
    per_core = [
        [np.asarray(m[name]) for name in in_names[:n_params]] for m in in_maps
    ]
    dev_in = [
        jax.device_put(
            np.concatenate([per_core[c][i] for c in range(n_cores)], axis=0),
            shd,
        )
        for i in range(n_params)
    ]
    n_calls = warmup + (iters if iters else 0)
    zsets = [
        [
            jax.device_put(
                np.zeros((n_cores * z.shape[0], *z.shape[1:]), z.dtype), shd
            )
            for z in zero_outs
        ]
        for _ in range(max(n_calls, 1))
    ]
    outs = None
    for i in range(warmup):
        outs = sharded(*dev_in, *zsets[i])
        jax.block_until_ready(outs)
    dt = None
    if iters:
        t0 = time.perf_counter()
        ress = [sharded(*dev_in, *zsets[warmup + i]) for i in range(iters)]
        jax.block_until_ready(ress)
        dt = (time.perf_counter() - t0) / iters
        outs = ress[-1]
    if outs is None:
        outs = sharded(*dev_in, *zsets[0])
    results = [
        {
            name: np.asarray(outs[i]).reshape(n_cores, *out_avals[i].shape)[c]
            for i, name in enumerate(out_names)
        }
        for c in range(n_cores)
    ]
    return results, dt


def run(inputs: dict, timed: bool = False):
    cfg = Cfg()
    x = np.asarray(inputs["x"])
    ei = np.asarray(inputs["edge_index"])
    batch = np.asarray(inputs["batch"])
    hd = prep_host(cfg, x, ei, batch, inputs)
    nc, input_names = build_program(cfg, hd, inputs)
    in_maps = make_in_maps(cfg, hd, inputs, input_names)
    results, dt = _run_sharded_timed(
        nc, in_maps, cfg.n_cores,
        iters=(10 if timed else 0), warmup=(2 if timed else 1),
    )
    outs = [results[c]["pooled"] for c in range(cfg.n_cores)]
    full = np.concatenate(outs, axis=0).astype(np.float32)
    return full, dt


def kernel(**inputs) -> np.ndarray:
    out, _ = run(inputs, timed=False)
    return out


# revision 17
# speedup vs baseline: 1.2309x; 1.2309x over previous
"""GIN (3-layer) message-passing kernel for Trainium2, 8 NeuronCores.

Strategy (graph-partition data parallel):
  - Graphs (variable 49-51 nodes, from the `batch` input) are padded to
    GP=52 columns each.  Core c owns graphs [750c, 750(c+1)), i.e. 39000
    padded columns (SHP=39040 with block padding); every graph lives on
    exactly one core so segment-max pooling is core-local.
  - Edges sharded by destination core.  Host sorts each core's edges by
    (512-column destination group, padded local dst) and pads each GROUP's
    edge list to a multiple of 128 ("k-tiles").  The k-tile structure is
    shared across cores (max over cores per group) so the SPMD program is
    identical on all cores.  Self-loops are NOT gathered: the GIN "+ h_i"
    term is added as a (pre-masked) column slab of the previous layer's z.
  - Aggregation: per-k-tile indirect-DMA gathers fetch h[src] rows (256B
    fp16) from a replicated DRAM table; a per-k-tile one-hot matrix S
    [128 x 512] (fp16, generated on the vector engine with iota/is_equal
    against rel = dst-in-group) right-multiplies the gathered tile on the
    tensor engine, accumulating aggT[feat, col] in a full-bank PSUM tile
    per group.
  - MLP runs in transposed space (feat on partitions).  BatchNorm of the
    previous layer is folded into the next layer's first matmul (w1
    row-scaled by s, plus a rank-1 (w1^T t) x (deg+1) correction), so h
    tables stay un-normalized.
  - Pad columns see zero aggregation and zero deg, so their z2 value is the
    input-independent constant v_l = relu(w2^T relu(b1) + b2).  BN stats
    (from activation accum_out, taken before masking) are corrected by
    npad*[v, v^2] before the 1KB AllReduce.  z2 is then multiplied by the
    0/1 column mask, so pad columns are exactly 0 in zT / z_rm / h_tbl --
    making the pooling max (z2 >= 0) and the next layer's self-term
    correct without further masking.
  - AllGather (fp16, 10MB -> 80MB) rebuilds the replicated h table between
    layers.
  - Pooling: per-layer local segment-max (fixed 52-wide windows) over the
    fp32 zT table, then the (monotone, gamma>0) BN affine, transpose,
    concat and per-core output.
Host assembles the 8 per-core [750, 384] outputs into the full [6000, 384].
"""

import sys

sys.path.insert(0, "/opt/trn_rl_repo")

from dataclasses import dataclass

import numpy as np


@dataclass(frozen=True)
class Cfg:
    n_nodes: int = 300000
    n_graphs: int = 6000
    n_cores: int = 8
    in_dim: int = 77
    dim: int = 128
    gp: int = 52  # padded columns per graph
    grp_blks: int = 4  # 128-col blocks per MLP/agg group (=512 cols)
    eps: float = 1e-5

    @property
    def gpc(self):  # graphs per core
        return self.n_graphs // self.n_cores

    @property
    def shc(self):  # padded columns per core (graph padding only)
        return self.gpc * self.gp

    @property
    def nb(self):  # 128-col blocks per core
        return (self.shc + 127) // 128

    @property
    def shp(self):  # block-padded columns per core
        return self.nb * 128

    @property
    def tbl(self):  # replicated table rows
        return self.n_cores * self.shp

    @property
    def ng(self):  # groups per core
        return (self.nb + self.grp_blks - 1) // self.grp_blks


@dataclass
class HostData:
    kt_total: int
    grp_kt0: np.ndarray  # [ng] int, first k-tile of each group
    grp_nk: np.ndarray  # [ng] int, k-tiles per group
    idx_sb: list  # per core [128, KT] int32 gather row ids
    rel_sb: list  # per core [128, KT] f32 dst-in-group (or -1 pad)
    deg2: list  # per core [128, ncolg*GW] f32 (packed rows 0/64)
    xT: list  # per core [128, SHP] f32 transposed padded x
    x_tbl: np.ndarray  # [tbl, 128] f16


def prep_host(cfg: Cfg, x: np.ndarray, edge_index: np.ndarray,
              batch: np.ndarray, inputs: dict) -> HostData:
    C, NG, SHP = cfg.n_cores, cfg.ng, cfg.shp
    N, GP, GPC = cfg.n_nodes, cfg.gp, cfg.gpc
    GW = cfg.grp_blks * 128
    batch = batch.astype(np.int64)
    sizes = np.bincount(batch, minlength=cfg.n_graphs)
    assert sizes.max() <= GP, f"graph size {sizes.max()} > pad {GP}"
    gstart = np.zeros(cfg.n_graphs + 1, dtype=np.int64)
    np.cumsum(sizes, out=gstart[1:])
    rank = np.arange(N, dtype=np.int64) - gstart[batch]
    padpos = (batch - (batch // GPC) * GPC) * GP + rank  # within-core col
    ncore = batch // GPC  # owning core of each node
    row = ncore * SHP + padpos  # replicated-table row

    src = edge_index[0].astype(np.int64)
    dst = edge_index[1].astype(np.int64)
    dcore = ncore[dst]
    dloc = padpos[dst]

    per_core = []
    cnts = np.zeros((C, NG), dtype=np.int64)
    for c in range(C):
        m = dcore == c
        s_c, dl_c = src[m], dloc[m]
        order = np.argsort(dl_c, kind="stable")
        s_c, dl_c = s_c[order], dl_c[order]
        grp = dl_c // GW
        cnts[c] = np.bincount(grp, minlength=NG)
        per_core.append((s_c, dl_c, grp))

    grp_nk = (cnts.max(axis=0) + 127) // 128  # shared k-tile structure
    grp_nk = np.maximum(grp_nk, 1)
    grp_kt0 = np.concatenate([[0], np.cumsum(grp_nk)[:-1]])
    KT = int(grp_nk.sum())
    k_pad = KT * 128

    idx_sb, rel_sb, deg2, xT = [], [], [], []
    for c in range(C):
        s_c, dl_c, grp = per_core[c]
        gs = np.concatenate([[0], np.cumsum(cnts[c])[:-1]])
        pos = np.arange(len(s_c)) - gs[grp]
        slot = grp_kt0[grp] * 128 + pos
        idx_arr = np.zeros(k_pad, dtype=np.int32)
        rel_arr = np.full(k_pad, -1.0, dtype=np.float32)
        idx_arr[slot] = row[s_c].astype(np.int32)
        rel_arr[slot] = (dl_c - grp * GW).astype(np.float32)
        idx_sb.append(np.ascontiguousarray(idx_arr.reshape(KT, 128).T))
        rel_sb.append(np.ascontiguousarray(rel_arr.reshape(KT, 128).T))

        own = ncore == c
        # in-degree + 1 (self) at padded cols; row g of deg2 = group g
        indeg = np.zeros(SHP, dtype=np.float32)
        np.add.at(indeg, padpos[dst[dcore == c]], 1.0)
        realm = np.zeros(SHP, dtype=np.float32)
        realm[padpos[own]] = 1.0
        deg_p = (indeg + 1.0) * realm
        ncolg = (NG + 2) // 3
        d2 = np.zeros((128, ncolg * GW), dtype=np.float32)
        for g in range(NG):
            sl = slice(g * GW, min((g + 1) * GW, SHP))
            n = sl.stop - sl.start
            d2[(g % 3) * 32, (g // 3) * GW: (g // 3) * GW + n] = deg_p[sl]
        deg2.append(d2)
        xt = np.zeros((128, SHP), dtype=np.float32)
        xt[: cfg.in_dim, padpos[own]] = x[own].T
        xT.append(np.ascontiguousarray(xt))

    x_tbl = np.zeros((cfg.tbl, 128), dtype=np.float16)
    x_tbl[row, : cfg.in_dim] = x.astype(np.float16)
    return HostData(KT, grp_kt0, grp_nk, idx_sb, rel_sb, deg2, xT, x_tbl)


def build_program(cfg: Cfg, hd: HostData, weights_np: dict):
    """Returns (nc, input_names).  weights_np supplies shapes only."""
    import concourse.bass as bass
    import concourse.mybir as mybir
    import concourse.tile as tile
    from concourse import bacc
    from concourse.masks import make_identity

    dt = mybir.dt
    Alu = mybir.AluOpType
    Act = mybir.ActivationFunctionType

    C, D, NB, SHP, TBL, NG, KT = (
        cfg.n_cores, cfg.dim, cfg.nb, cfg.shp, cfg.tbl, cfg.ng, hd.kt_total,
    )
    GW = cfg.grp_blks * 128  # group width (cols)
    GS, GPC = cfg.gp, cfg.gpc
    inv_n = 1.0 / cfg.n_nodes

    nc = bacc.Bacc(
        "TRN2", target_bir_lowering=False, debug=False, num_devices=C
    )

    def din(name, shape, dtp=dt.float32):
        return nc.dram_tensor(name, list(shape), dtp, kind="ExternalInput").ap()

    x_tbl_d = din("x_tbl", (TBL, D), dt.float16)
    idx_d = din("idx", (128, KT), dt.int32)
    rel_d = din("rel", (128, KT))
    ncolg = (NG + 2) // 3
    deg2_d = din("deg2", (128, ncolg * GW))
    xT_d = din("xT", (128, SHP))
    iota_d = din("iota", (128, GW), dt.float16)
    w1_d = [din(f"w1_{l}", (D, D)) for l in range(3)]
    w2_d = [din(f"w2_{l}", (D, D)) for l in range(3)]
    b1_d = [din(f"b1_{l}", (D, 1)) for l in range(3)]
    b2_d = [din(f"b2_{l}", (D, 1)) for l in range(3)]
    gb_d = din("gb", (D, 6))  # cols: g0 b0 g1 b1 g2 b2
    out_d = nc.dram_tensor(
        "pooled", [GPC, 3 * D], dt.float32, kind="ExternalOutput"
    ).ap()

    input_names = (
        ["x_tbl", "idx", "rel", "deg2", "xT", "iota"]
        + [f"w1_{l}" for l in range(3)]
        + [f"w2_{l}" for l in range(3)]
        + [f"b1_{l}" for l in range(3)]
        + [f"b2_{l}" for l in range(3)]
        + ["gb"]
    )

    n_pool_chunks = (GPC + 127) // 128
    last_chunk_rows = GPC - (n_pool_chunks - 1) * 128
    # pooling slab: whole graphs per slab, tile cols = graphs*GS
    slab_graphs = max(1, 2496 // GS)
    n_slabs = (GPC + slab_graphs - 1) // slab_graphs

    with tile.TileContext(nc) as tc:
        with (
            tc.tile_pool(name="const", bufs=1) as cpool,
            tc.tile_pool(name="ebuf", bufs=8) as epool,
            tc.tile_pool(name="spool", bufs=4) as spool,
            tc.tile_pool(name="zin", bufs=2) as zinpool,
            tc.tile_pool(name="zmid", bufs=2) as zmidpool,
            tc.tile_pool(name="slfp", bufs=2) as slfpool,
            tc.tile_pool(name="rm", bufs=3) as rmpool,
            tc.tile_pool(name="stat", bufs=1) as statpool,
            tc.tile_pool(name="slab", bufs=2) as slabpool,
            tc.tile_pool(name="poolt", bufs=2) as ptpool,
            tc.tile_pool(name="agg_ps", bufs=2, space="PSUM") as aggpool,
            tc.tile_pool(name="m1_ps", bufs=2, space="PSUM") as m1pool,
            tc.tile_pool(name="m2_ps", bufs=2, space="PSUM") as m2pool,
            tc.tile_pool(name="tr_ps", bufs=2, space="PSUM") as trpool,
            tc.tile_pool(name="dram", bufs=1, space="DRAM") as dpool,
        ):
            # ---- DRAM intermediates ----
            h_tbl = [
                dpool.tile([TBL, D], dt.float16, name=f"h_tbl{l}")
                for l in range(2)
            ]
            z_rm = dpool.tile([SHP, D], dt.float16, name="z_rm")
            zT = [
                dpool.tile([D, SHP], dt.float32, name=f"zT{l}") for l in range(3)
            ]
            st_in = [
                dpool.tile([D, 2], dt.float32, name=f"st_in{l}") for l in range(3)
            ]
            st_out = [
                dpool.tile([D, 2], dt.float32, name=f"st_out{l}")
                for l in range(3)
            ]

            # ---- constants to SBUF ----
            def load(shape, src_ap, dtp=dt.float32, name=None):
                t = cpool.tile(list(shape), dtp, name=name)
                nc.sync.dma_start(out=t[:], in_=src_ap)
                return t

            idx_sb = load((128, KT), idx_d[:], dt.int32, name="idx_sb")
            rel_sb = load((128, KT), rel_d[:], name="rel_sb")
            deg2_sb = load((128, ncolg * GW), deg2_d[:], name="deg2_sb")
            # pad-col indicator: deg2 is deg+1>=1 at real cols, 0 at pads
            padm2_sb = cpool.tile([128, ncolg * GW], dt.float32, name="padm2_sb")
            nc.vector.tensor_scalar(
                out=padm2_sb[:], in0=deg2_sb[:], scalar1=0.0, scalar2=None,
                op0=Alu.is_equal,
            )
            iota_sb = load((128, GW), iota_d[:], dt.float16, name="iota_sb")
            w1_sb = [load((D, D), w1_d[l][:], name=f"w1sb{l}") for l in range(3)]
            w2_sb = [load((D, D), w2_d[l][:], name=f"w2sb{l}") for l in range(3)]
            b1_sb = [load((D, 1), b1_d[l][:], name=f"b1sb{l}") for l in range(3)]
            b2_sb = [load((D, 1), b2_d[l][:], name=f"b2sb{l}") for l in range(3)]
            gb_sb = load((D, 6), gb_d[:], name="gb_sb")
            ident = cpool.tile([128, 128], dt.float32, name="ident")
            make_identity(nc, ident[:])

            # persistent small tiles
            s_all = cpool.tile([D, 3], dt.float32, name="s_all")
            t_all = cpool.tile([D, 3], dt.float32, name="t_all")
            w1s_sb = [
                cpool.tile([D, D], dt.float32, name=f"w1s{l}") for l in (1, 2)
            ]
            u_sb = [cpool.tile([1, D], dt.float32, name=f"u{l}") for l in (1, 2)]
            ub_sb = [
                cpool.tile([D, D], dt.float32, name=f"ub{l}") for l in (1, 2)
            ]
            ones_row = cpool.tile([1, D], dt.float32, name="ones_row")
            nc.gpsimd.memset(ones_row[:], 1.0)
            negbig = cpool.tile([128, D], dt.float32, name="negbig")
            nc.gpsimd.memset(negbig[:], -1e30)
            ssum = cpool.tile([128, NG], dt.float32, name="ssum")
            ssq = cpool.tile([128, NG], dt.float32, name="ssq")
            sq_scr = cpool.tile([128, GW], dt.float32, name="sq_scr")
            stat_scr = cpool.tile([128, 8], dt.float32, name="stat_scr")

            def compute_fold(l):
                """Load layer-l AR'd stats; fill s_all/t_all col l and (for
                l<2) w1s_sb/u_sb of layer l+1."""
                st = statpool.tile([D, 2], dt.float32, name="st_ld")
                nc.sync.dma_start(out=st[:], in_=st_out[l][:])
                mu = stat_scr[:, 0:1]
                msq = stat_scr[:, 1:2]
                var = stat_scr[:, 2:3]
                rstd = stat_scr[:, 3:4]
                smu = stat_scr[:, 4:5]
                nc.vector.tensor_scalar_mul(mu, st[:, 0:1], inv_n)
                nc.vector.tensor_scalar_mul(msq, st[:, 1:2], inv_n)
                nc.vector.tensor_tensor(
                    out=var, in0=mu, in1=mu, op=Alu.mult
                )
                nc.vector.tensor_tensor(
                    out=var, in0=msq, in1=var, op=Alu.subtract
                )
                veps = stat_scr[:, 6:7]
                nc.vector.tensor_scalar_add(veps, var, cfg.eps)
                std = stat_scr[:, 5:6]
                nc.scalar.activation(std, veps, Act.Sqrt)
                nc.vector.reciprocal(rstd, std)
                scol = s_all[:, l : l + 1]
                tcol = t_all[:, l : l + 1]
                nc.vector.tensor_tensor(
                    out=scol, in0=gb_sb[:, 2 * l : 2 * l + 1], in1=rstd,
                    op=Alu.mult,
                )
                nc.vector.tensor_tensor(out=smu, in0=scol, in1=mu, op=Alu.mult)
                nc.vector.tensor_tensor(
                    out=tcol, in0=gb_sb[:, 2 * l + 1 : 2 * l + 2], in1=smu,
                    op=Alu.subtract,
                )
                if l < 2:
                    ln = l + 1
                    nc.vector.tensor_scalar(
                        out=w1s_sb[ln - 1][:], in0=w1_sb[ln][:], scalar1=scol,
                        scalar2=None, op0=Alu.mult,
                    )
                    ups = trpool.tile([1, D], dt.float32, name="ups", tag="tr")
                    nc.tensor.matmul(
                        ups[:], lhsT=tcol, rhs=w1_sb[ln][:], start=True,
                        stop=True,
                    )
                    nc.any.tensor_copy(out=u_sb[ln - 1][:], in_=ups[:])
                    ubp = trpool.tile([D, D], dt.float32, name="ubp", tag="tr")
                    nc.tensor.matmul(
                        ubp[:], lhsT=ones_row[:], rhs=u_sb[ln - 1][:],
                        start=True, stop=True,
                    )
                    nc.any.tensor_copy(out=ub_sb[ln - 1][:], in_=ubp[:])

            out_big = cpool.tile(
                [128, n_pool_chunks * 3 * D], dt.float32, name="out_big"
            )

            def emit_pool(l):
                """Segment-max pool layer l's zT, BN-affine, transpose into
                out_big.  Requires compute_fold(l) done (s_all/t_all col l)."""
                pt = ptpool.tile([128, GPC], dt.float32, name="pt")
                for si in range(n_slabs):
                    g0 = si * slab_graphs
                    g1 = min(g0 + slab_graphs, GPC)
                    ncols = (g1 - g0) * GS
                    sl = slabpool.tile(
                        [128, slab_graphs * GS], dt.float32, name="sl"
                    )
                    nc.sync.dma_start(
                        out=sl[:, :ncols],
                        in_=zT[l][:, g0 * GS : g0 * GS + ncols],
                    )
                    nc.vector.tensor_reduce(
                        out=pt[:, g0:g1],
                        in_=sl[:, :ncols].rearrange("p (g s) -> p g s", s=GS),
                        axis=mybir.AxisListType.X, op=Alu.max,
                    )
                pta = ptpool.tile([128, GPC], dt.float32, name="pta")
                nc.vector.tensor_scalar(
                    out=pta[:], in0=pt[:], scalar1=s_all[:, l : l + 1],
                    scalar2=t_all[:, l : l + 1], op0=Alu.mult, op1=Alu.add,
                )
                for ch in range(n_pool_chunks):
                    rows = 128 if ch < n_pool_chunks - 1 else last_chunk_rows
                    trp = trpool.tile(
                        [128, 128], dt.float32, name="trpo", tag="tr"
                    )
                    nc.tensor.transpose(
                        trp[:rows, :], pta[:, ch * 128 : ch * 128 + rows],
                        ident[:],
                    )
                    nc.any.tensor_copy(
                        out=out_big[
                            :rows,
                            ch * 3 * D + l * D : ch * 3 * D + (l + 1) * D,
                        ],
                        in_=trp[:rows, :],
                    )

            for layer in range(3):
                tbl_ap = x_tbl_d if layer == 0 else h_tbl[layer - 1][:]
                if layer > 0:
                    compute_fold(layer - 1)
                    emit_pool(layer - 1)
                lhs1 = w1_sb[0] if layer == 0 else w1s_sb[layer - 1]

                for g in range(NG):
                    W = min(GW, SHP - g * GW)
                    nk = int(hd.grp_nk[g])
                    t0 = int(hd.grp_kt0[g])
                    agg = aggpool.tile([128, GW], dt.float32, name="agg")
                    for j in range(nk):
                        t = t0 + j
                        esl = epool.tile([128, 128], dt.float16, name="ebuf")
                        nc.gpsimd.indirect_dma_start(
                            out=esl[:],
                            out_offset=None,
                            in_=tbl_ap,
                            in_offset=bass.IndirectOffsetOnAxis(
                                ap=idx_sb[:, t : t + 1], axis=0,
                            ),
                        )
                        s_t = spool.tile([128, GW], dt.float16, name="s_t")
                        nc.vector.tensor_scalar(
                            out=s_t[:, :W], in0=iota_sb[:, :W],
                            scalar1=rel_sb[:, t : t + 1],
                            scalar2=None, op0=Alu.is_equal,
                        )
                        nc.tensor.matmul(
                            agg[:, :W], lhsT=esl[:], rhs=s_t[:, :W],
                            start=(j == 0), stop=(j == nk - 1),
                        )
                    # ---- self term: zin = agg + h_prev (pre-masked) ----
                    zin = zinpool.tile([128, GW], dt.float32, name="zin")
                    slf = slfpool.tile([128, GW], dt.float32, name="slf")
                    if layer == 0:
                        nc.sync.dma_start(
                            out=slf[:, :W], in_=xT_d[:, g * GW : g * GW + W]
                        )
                    else:
                        nc.sync.dma_start(
                            out=slf[:, :W],
                            in_=zT[layer - 1][:, g * GW : g * GW + W],
                        )
                    nc.vector.tensor_tensor(
                        out=zin[:, :W], in0=agg[:, :W], in1=slf[:, :W],
                        op=Alu.add,
                    )
                    # ---- MLP on the group (transposed space) ----
                    m1 = m1pool.tile([128, GW], dt.float32, name="m1")
                    nc.tensor.matmul(
                        m1[:, :W], lhsT=lhs1[:], rhs=zin[:, :W],
                        start=True, stop=(layer == 0),
                    )
                    if layer > 0:
                        dp1 = (g % 3) * 32
                        dc1 = (g // 3) * GW
                        nc.tensor.matmul(
                            m1[:, :W], lhsT=ub_sb[layer - 1][dp1 : dp1 + 1, :],
                            rhs=deg2_sb[dp1 : dp1 + 1, dc1 : dc1 + W],
                            start=False, stop=True,
                        )
                    z1 = zmidpool.tile([128, GW], dt.float32, name="z1")
                    nc.scalar.activation(
                        z1[:, :W], m1[:, :W], Act.Relu, bias=b1_sb[layer][:]
                    )
                    m2 = m2pool.tile([128, GW], dt.float32, name="m2")
                    nc.tensor.matmul(
                        m2[:, :W], lhsT=w2_sb[layer][:], rhs=z1[:, :W],
                        start=True, stop=False,
                    )
                    dp = (g % 3) * 32
                    dc = (g // 3) * GW
                    nc.tensor.matmul(
                        m2[:, :W], lhsT=negbig[dp : dp + 1, :],
                        rhs=padm2_sb[dp : dp + 1, dc : dc + W],
                        start=False, stop=True,
                    )
                    z2 = zmidpool.tile([128, GW], dt.float32, name="z2")
                    nc.scalar.activation(
                        z2[:, :W], m2[:, :W], Act.Relu,
                        bias=b2_sb[layer][:], accum_out=ssum[:, g : g + 1],
                    )
                    nc.scalar.activation(
                        sq_scr[:, :W], z2[:, :W], Act.Square,
                        accum_out=ssq[:, g : g + 1],
                    )
                    nc.sync.dma_start(
                        out=zT[layer][:, g * GW : g * GW + W], in_=z2[:, :W]
                    )
                    if layer < 2:
                        rm = rmpool.tile([128, GW], dt.float16, name="rm")
                        for i in range(W // 128):
                            trp = trpool.tile(
                                [128, 128], dt.float32, name="trp", tag="tr"
                            )
                            nc.tensor.transpose(
                                trp[:], z2[:, i * 128 : (i + 1) * 128],
                                ident[:],
                            )
                            nc.any.tensor_copy(
                                out=rm[:, i * 128 : (i + 1) * 128], in_=trp[:]
                            )
                        nc.sync.dma_start(
                            out=z_rm[g * GW : g * GW + W, :].rearrange(
                                "(k p) d -> p k d", p=128
                            ),
                            in_=rm[:, :W].rearrange("p (k d) -> p k d", d=D),
                        )

                # ---- stats reduce (pad-corrected) + AllReduce ----
                sp = statpool.tile([D, 2], dt.float32, name="sp")
                nc.vector.tensor_reduce(
                    out=sp[:, 0:1], in_=ssum[:, :NG],
                    axis=mybir.AxisListType.X, op=Alu.add,
                )
                nc.vector.tensor_reduce(
                    out=sp[:, 1:2], in_=ssq[:, :NG],
                    axis=mybir.AxisListType.X, op=Alu.add,
                )
                nc.sync.dma_start(out=st_in[layer][:], in_=sp[:])
                nc.gpsimd.collective_compute(
                    "AllReduce", Alu.add,
                    replica_groups=[list(range(C))],
                    ins=[st_in[layer].opt()], outs=[st_out[layer].opt()],
                )
                if layer < 2:
                    nc.gpsimd.collective_compute(
                        "AllGather", Alu.bypass,
                        replica_groups=[list(range(C))],
                        ins=[z_rm.opt()], outs=[h_tbl[layer].opt()],
                    )

            # ---- final fold + pool of layer 2, then output ----
            compute_fold(2)
            emit_pool(2)
            for ch in range(n_pool_chunks):
                rows = 128 if ch < n_pool_chunks - 1 else last_chunk_rows
                nc.sync.dma_start(
                    out=out_d[ch * 128 : ch * 128 + rows, :],
                    in_=out_big[:rows, ch * 3 * D : (ch + 1) * 3 * D],
                )

    nc.compile()
    return nc, input_names


def make_in_maps(cfg: Cfg, hd: HostData, inputs: dict, input_names):
    GW = cfg.grp_blks * 128
    # column index 0..GW-1 (dst-in-group), same on every partition
    iota = np.tile(np.arange(GW, dtype=np.float16), (128, 1))
    gb = np.zeros((cfg.dim, 6), dtype=np.float32)
    for l in range(3):
        gb[:, 2 * l] = inputs["gamma"][l]
        gb[:, 2 * l + 1] = inputs["beta"][l]
    w1p = []
    for l in range(3):
        w = np.zeros((cfg.dim, cfg.dim), dtype=np.float32)
        wl = inputs[f"w1_{l}"]
        w[: wl.shape[0], :] = wl
        w1p.append(w)
    shared = {
        "x_tbl": hd.x_tbl,
        "iota": np.ascontiguousarray(iota),
        "gb": gb,
    }
    for l in range(3):
        shared[f"w1_{l}"] = w1p[l]
        shared[f"w2_{l}"] = np.ascontiguousarray(inputs[f"w2_{l}"].astype(np.float32))
        shared[f"b1_{l}"] = inputs[f"b1_{l}"].astype(np.float32).reshape(-1, 1)
        shared[f"b2_{l}"] = inputs[f"b2_{l}"].astype(np.float32).reshape(-1, 1)
    in_maps = []
    for c in range(cfg.n_cores):
        m = dict(shared)
        m["idx"] = hd.idx_sb[c]
        m["rel"] = hd.rel_sb[c]
        m["deg2"] = hd.deg2[c]
        m["xT"] = hd.xT[c]
        assert set(m.keys()) == set(input_names)
        in_maps.append(m)
    return in_maps


def _run_sharded_timed(nc, in_maps, n_cores, iters=10, warmup=2):
    """Execute the compiled Bass module via PJRT with device-resident inputs,
    timing `iters` back-to-back dispatches (excludes input upload/compile)."""
    import time

    import jax
    from jax.sharding import Mesh, NamedSharding, PartitionSpec
    from jax.experimental.shard_map import shard_map

    import concourse.mybir as mybir
    from concourse import bass2jax

    bass2jax.install_neuronx_cc_hook()
    partition_name = (
        nc.partition_id_tensor.name if nc.partition_id_tensor else None
    )
    in_names, out_names, out_avals, zero_outs = [], [], [], []
    for alloc in nc.m.functions[0].allocations:
        if not isinstance(alloc, mybir.MemoryLocationSet):
            continue
        name = alloc.memorylocations[0].name
        if alloc.kind == "ExternalInput":
            if name != partition_name:
                in_names.append(name)
        elif alloc.kind == "ExternalOutput":
            out_names.append(name)
            shape = tuple(alloc.tensor_shape)
            dtp = mybir.dt.np(alloc.dtype)
            out_avals.append(jax.core.ShapedArray(shape, dtp))
            zero_outs.append(np.zeros(shape, dtp))
    n_params, n_outs = len(in_names), len(out_avals)
    in_names.extend(out_names)
    if partition_name is not None:
        in_names.append(partition_name)
    donate = tuple(range(n_params, n_params + n_outs))

    def _body(*args):
        operands = list(args)
        if partition_name is not None:
            operands.append(bass2jax.partition_id_tensor())
        outs = bass2jax._bass_exec_p.bind(
            *operands,
            out_avals=tuple(out_avals),
            in_names=tuple(in_names),
            out_names=tuple(out_names),
            lowering_input_output_aliases=(),
            sim_require_finite=True,
            sim_require_nnan=True,
            nc=nc,
        )
        return tuple(outs)

    devices = jax.devices()[:n_cores]
    mesh = Mesh(np.asarray(devices), ("core",))
    pspec = PartitionSpec("core")
    in_specs = (pspec,) * (n_params + n_outs)
    sharded = jax.jit(
        shard_map(
            _body, mesh=mesh, in_specs=in_specs,
            out_specs=(pspec,) * len(out_names), check_rep=False,
        ),
        donate_argnums=donate, keep_unused=True,
    )
    shd = NamedSharding(mesh, pspec)
    per_core = [
        [np.asarray(m[name]) for name in in_names[:n_params]] for m in in_maps
    ]
    dev_in = [
        jax.device_put(
            np.concatenate([per_core[c][i] for c in range(n_cores)], axis=0),
            shd,
        )
        for i in range(n_params)
    ]
    n_calls = warmup + (iters if iters else 0)
    zsets = [
        [
            jax.device_put(
                np.zeros((n_cores * z.shape[0], *z.shape[1:]), z.dtype), shd
            )
            for z in zero_outs
        ]
        for _ in range(max(n_calls, 1))
    ]
    outs = None
    for i in range(warmup):
        outs = sharded(*dev_in, *zsets[i])
        jax.block_until_ready(outs)
    dt = None
    if iters:
        t0 = time.perf_counter()
        ress = [sharded(*dev_in, *zsets[warmup + i]) for i in range(iters)]
        jax.block_until_ready(ress)
        dt = (time.perf_counter() - t0) / iters
        outs = ress[-1]
    if outs is None:
        outs = sharded(*dev_in, *zsets[0])
    results = [
        {
            name: np.asarray(outs[i]).reshape(n_cores, *out_avals[i].shape)[c]
            for i, name in enumerate(out_names)
        }
        for c in range(n_cores)
    ]
    return results, dt


def run(inputs: dict, timed: bool = False):
    cfg = Cfg()
    x = np.asarray(inputs["x"])
    ei = np.asarray(inputs["edge_index"])
    batch = np.asarray(inputs["batch"])
    hd = prep_host(cfg, x, ei, batch, inputs)
    nc, input_names = build_program(cfg, hd, inputs)
    in_maps = make_in_maps(cfg, hd, inputs, input_names)
    results, dt = _run_sharded_timed(
        nc, in_maps, cfg.n_cores,
        iters=(10 if timed else 0), warmup=(2 if timed else 1),
    )
    outs = [results[c]["pooled"] for c in range(cfg.n_cores)]
    full = np.concatenate(outs, axis=0).astype(np.float32)
    return full, dt


def kernel(**inputs) -> np.ndarray:
    out, _ = run(inputs, timed=False)
    return out
